# revision 2
# baseline (speedup 1.0000x reference)
"""Trainium2 Bass kernel for the MAMGCN encoder block.

Data-parallel over batch B=16 across 8 NeuronCores (2 batches/core).

Device pipeline per batch:
  x -> attention pre-reductions -> product -> P=tanh(.5(prod+bs))
    -> S_pre=Vs@P -> E=exp(.5 S_pre + hrow) -> colsums -> rT=1/csum
  Y[n,k,t,o] = x @ Theta (block-diag matmul, full T resident)
  conv per m-chunk: A-tiles = cheb_mc * E computed on the fly from an
  m-major cheb layout (one contiguous DMA per m-chunk), accumulate
  po = sum_{k,cn} A^T Y over T-halves, relu(rT*po^T) -> out.

Differences vs the staged baseline:
  * 512-wide PSUM accumulation chains (1 bank/slot) so product/VsP/exp
    pipeline without draining full-1024 tiles; PSUM fits in 8 banks with
    double buffering everywhere.
  * Y built once per batch (full T) with a single strided DVE copy per
    (j,cn) tile instead of two copies.
  * A = cheb*E computed per m-chunk on the fly (no 48KB A buffer), cheb
    streamed in an m-major layout with large contiguous descriptors.
  * x double-buffered in j-halves for cross-batch DMA overlap.
"""
import numpy as np
import ml_dtypes

B, N, F, T, K, FO = 16, 1024, 64, 24, 3, 64
NCORES = 8
BPC = B // NCORES          # batches per core
NCH = N // 128             # 8 partition chunks of N
NJ = (T * F) // 128        # 12 chunks of the tf dim
THALF = T // 2             # 12
bf16 = ml_dtypes.bfloat16

_CACHE = {}


def _build_nc(reps=1, iadd=True, py2=True):
    import contextlib
    import concourse.bacc as bacc
    import concourse.tile as tile
    import concourse.mybir as mybir

    fp32 = mybir.dt.float32
    bf = mybir.dt.bfloat16
    AF = mybir.ActivationFunctionType

    nc = bacc.Bacc(
        "TRN2", target_bir_lowering=False, debug=False,
        num_devices=NCORES,
    )

    # ---- DRAM I/O ----
    x_d = nc.dram_tensor("x_tf", [BPC, NJ, 128, N], bf, kind="ExternalInput")
    bs_d = nc.dram_tensor("bs_t", [NCH, 128, N], bf, kind="ExternalInput")
    vs_d = nc.dram_tensor("vs_t", [NCH, 128, N], bf, kind="ExternalInput")
    # m-major cheb: [mc, 128(n%128), k, cn(n//128), 128(m%128)]
    chebm_d = nc.dram_tensor("cheb_m", [NCH, 128, K * NCH * 128], bf,
                             kind="ExternalInput")
    wcat_d = nc.dram_tensor("wcat", [NJ, 128, 2 * T], bf, kind="ExternalInput")
    th2_d = nc.dram_tensor("th2", [128, 2 * K * FO], bf, kind="ExternalInput")
    hrow_d = nc.dram_tensor("hrow", [NCH, 128, 1], fp32, kind="ExternalInput")
    ident_d = nc.dram_tensor("ident", [128, 128], bf, kind="ExternalInput")
    # out[b, half, mchunk, p, o, tl]
    out_d = nc.dram_tensor("out", [BPC, 2, NCH, 128, FO, THALF], fp32,
                           kind="ExternalOutput")

    with tile.TileContext(nc) as tc:
      with (tc.For_i(0, reps, 1) if reps > 1 else contextlib.nullcontext()):
        with (
            tc.tile_pool(name="const", bufs=1) as cpool,
            tc.tile_pool(name="xp", bufs=1) as xpool,
            tc.tile_pool(name="work", bufs=2) as wpool,
            tc.tile_pool(name="big", bufs=1) as bpool,
            tc.tile_pool(name="psA", bufs=2, space="PSUM") as psA,
            tc.tile_pool(name="psB", bufs=2, space="PSUM") as psB,
        ):
            # ---- constants (~34 KB/part) ----
            vsT_sb = cpool.tile([128, NCH, N], bf, tag="vsT")
            bs_sb = cpool.tile([128, NCH, N], bf, tag="bs")
            wcat_sb = cpool.tile([128, NJ, 2 * T], bf, tag="wcat")
            th2_sb = cpool.tile([128, 2 * K * FO], bf, tag="th2")
            hrow_sb = cpool.tile([128, NCH], fp32, tag="hrow")
            ones_sb = cpool.tile([128, 1], bf, tag="ones")
            one1_sb = cpool.tile([1, 1], fp32, tag="one1")
            ident_sb = cpool.tile([128, 128], bf, tag="ident")
            for j in range(NJ):
                nc.sync.dma_start(wcat_sb[:, j, :], wcat_d[j])
            nc.sync.dma_start(th2_sb[:], th2_d[:])
            nc.sync.dma_start(ident_sb[:], ident_d[:])
            for c in range(NCH):
                nc.sync.dma_start(hrow_sb[:, c:c + 1], hrow_d[c])
            nc.gpsimd.memset(ones_sb[:], 1.0)
            nc.gpsimd.memset(one1_sb[:], 1.0)

            for b in range(BPC):
                # ---- x load in two j-halves (24 KB/part total) ----
                xh = []
                for half in range(2):
                    xt = xpool.tile([128, NJ // 2, N], bf, tag=f"x{half}")
                    for j0 in range(0, NJ // 2, 3):
                        nc.sync.dma_start(
                            xt[:, j0:j0 + 3, :],
                            x_d[b, half * 6 + j0:half * 6 + j0 + 3]
                            .rearrange("j p n -> p j n"))
                    xh.append(xt)
                if b == 0:
                    for c in range(NCH):
                        nc.sync.dma_start(bs_sb[:, c, :], bs_d[c])
                        nc.sync.dma_start(vsT_sb[:, c, :], vs_d[c])

                def xj(j):
                    return xh[j // 6][:, j % 6, :]

                # ---- per-batch big tiles (~104 KB/part) ----
                e_sb = bpool.tile([128, NCH, N], bf, tag="e")
                p_sb = bpool.tile([128, NCH, N], bf, tag="p")
                y_sb = bpool.tile([128, NCH, K, T, FO], bf, tag="y")
                rT_sb = bpool.tile([128, NCH], fp32, tag="rT")

                # ---- attention pre-reductions (one pass over x) ----
                att_c = wpool.tile([2 * T, N], bf, tag="attc", bufs=1)
                att_r = wpool.tile([T, N], bf, tag="attr", bufs=1)
                for s in range(2):
                    pa = psA.tile([2 * T, 512], fp32, tag="A")
                    for j in range(NJ):
                        nc.tensor.matmul(
                            pa[:],
                            wcat_sb[:, j, :],
                            xj(j)[:, s * 512:(s + 1) * 512],
                            start=(j == 0), stop=(j == NJ - 1),
                        )
                    nc.scalar.copy(att_c[:, s * 512:(s + 1) * 512], pa[:])
                # shift rows 24..47 down to partitions 0..23 for the product
                nc.sync.dma_start(att_r[:], att_c[T:2 * T, :])
                att_l = att_c

                # ---- product + bs -> tanh(0.5*) -> P ----
                for cn in range(NCH):
                    for s in range(2):
                        pp = psA.tile([128, 512], fp32, tag="A")
                        nc.tensor.matmul(
                            pp[:],
                            att_l[0:T, cn * 128:(cn + 1) * 128],
                            att_r[:, s * 512:(s + 1) * 512],
                            start=True, stop=not iadd,
                        )
                        if iadd:
                            nc.tensor.matmul(
                                pp[:],
                                ident_sb[:],
                                bs_sb[:, cn, s * 512:(s + 1) * 512],
                                start=False, stop=True,
                            )
                            nc.scalar.activation(
                                p_sb[:, cn, s * 512:(s + 1) * 512],
                                pp[:], AF.Tanh, scale=0.5)
                        else:
                            tmp = wpool.tile([128, 512], bf, tag="tmp")
                            nc.vector.tensor_add(
                                tmp[:], pp[:],
                                bs_sb[:, cn, s * 512:(s + 1) * 512])
                            nc.scalar.activation(
                                p_sb[:, cn, s * 512:(s + 1) * 512],
                                tmp[:], AF.Tanh, scale=0.5)

                # ---- S_pre = Vs @ P (per i-chunk), exp -> E ----
                for ic in range(NCH):
                    for s in range(2):
                        ps = psA.tile([128, 512], fp32, tag="A")
                        for kc in range(NCH):
                            nc.tensor.matmul(
                                ps[:],
                                vsT_sb[:, kc, ic * 128:(ic + 1) * 128],
                                p_sb[:, kc, s * 512:(s + 1) * 512],
                                start=(kc == 0), stop=(kc == NCH - 1),
                            )
                        nc.scalar.activation(
                            e_sb[:, ic, s * 512:(s + 1) * 512], ps[:], AF.Exp,
                            scale=0.5, bias=hrow_sb[:, ic:ic + 1],
                        )

                # ---- column sums of E -> recip -> rT (128, 8) ----
                csum_sb = wpool.tile([1, N], fp32, tag="csum", bufs=1)
                for s in range(2):
                    pc = psA.tile([1, 512], fp32, tag="A")
                    for ic in range(NCH):
                        nc.tensor.matmul(
                            pc[:],
                            ones_sb[:],
                            e_sb[:, ic, s * 512:(s + 1) * 512],
                            start=(ic == 0), stop=(ic == NCH - 1),
                        )
                    nc.scalar.copy(csum_sb[:, s * 512:(s + 1) * 512], pc[:])
                prt = psA.tile([128, NCH], fp32, tag="A")
                for c in range(NCH):
                    nc.tensor.matmul(
                        prt[:, c:c + 1],
                        csum_sb[:, c * 128:(c + 1) * 128],
                        one1_sb[:],
                        start=True, stop=True,
                    )
                nc.vector.reciprocal(rT_sb[:], prt[:])

                # ---- Y build: y[cn, k, t, o] = x @ th2 (full T) ----
                # one strided copy per (j, cn), alternating DVE/ACT
                for j in range(NJ):
                    tl0 = 2 * j
                    for cn in range(NCH):
                        py = psA.tile([128, 2, K, FO], fp32, tag="A")
                        nc.tensor.matmul(
                            py[:, :, :, :],
                            xj(j)[:, cn * 128:(cn + 1) * 128],
                            th2_sb[:],
                            start=True, stop=True,
                        )
                        dst = y_sb[:, cn, :, tl0:tl0 + 2, :]
                        src_ap = py.rearrange("p t k o -> p k t o")
                        if cn % 2 == 0:
                            nc.vector.tensor_copy(dst, src_ap)
                        else:
                            nc.scalar.copy(dst, src_ap)

                # ---- graph conv per m-chunk ----
                for mc in range(NCH):
                    chm = wpool.tile([128, K, NCH, 128], bf, tag="chm")
                    nc.sync.dma_start(
                        chm.rearrange("p k c m -> p (k c m)"), chebm_d[mc])
                    am = wpool.tile([128, K, NCH, 128], bf, tag="am")
                    for k in range(K):
                        nc.vector.tensor_mul(
                            am[:, k, :, :],
                            chm[:, k, :, :],
                            e_sb[:, :, mc * 128:(mc + 1) * 128])
                    for h in range(2):
                        po = psB.tile([128, THALF, FO], fp32, tag="B")
                        nmm = 0
                        for k in range(K):
                            for cn in range(NCH):
                                first = nmm == 0
                                last = nmm == K * NCH - 1
                                t0 = h * THALF
                                nc.tensor.matmul(
                                    po[:, 0:8, :],
                                    am[:, k, cn, :],
                                    y_sb[:, cn, k, t0:t0 + 8, :],
                                    start=first, stop=last,
                                )
                                nc.tensor.matmul(
                                    po[:, 8:THALF, :],
                                    am[:, k, cn, :],
                                    y_sb[:, cn, k, t0 + 8:t0 + THALF, :],
                                    start=first, stop=last,
                                )
                                nmm += 1
                        st = wpool.tile([128, FO, THALF], fp32, tag="stage",
                                        bufs=3)
                        nc.scalar.activation(
                            st[:],
                            po[:, :, :].rearrange("p t o -> p o t"),
                            AF.Relu,
                            scale=rT_sb[:, mc:mc + 1],
                        )
                        nc.sync.dma_start(out_d[b, h, mc], st[:])

    nc.compile()
    return nc


def _host_prep(x, W1, W2, W3, bs, Vs, cheb, Theta):
    x = np.asarray(x, np.float32)
    W1 = np.asarray(W1, np.float32)
    W2 = np.asarray(W2, np.float32)
    W3 = np.asarray(W3, np.float32)
    bs = np.asarray(bs, np.float32)
    Vs = np.asarray(Vs, np.float32)
    cheb = np.asarray(cheb, np.float32)
    Theta = np.asarray(Theta, np.float32)

    x_tf = np.ascontiguousarray(x.transpose(0, 3, 2, 1)).reshape(B, NJ, 128, N)
    x_tf = x_tf.astype(bf16)
    bs_t = bs[0].reshape(NCH, 128, N).astype(bf16)
    vs_t = np.ascontiguousarray(Vs.T).reshape(NCH, 128, N).astype(bf16)
    # m-major cheb: cheb_m[mc, p, k, cn, m] = cheb[k, cn*128+p, mc*128+m]
    cm = cheb.reshape(K, NCH, 128, NCH, 128)          # k, cn, p, mc, m
    cheb_m = np.ascontiguousarray(cm.transpose(3, 2, 0, 1, 4)).reshape(
        NCH, 128, K * NCH * 128).astype(bf16)
    t_idx = np.arange(T * F) // F
    f_idx = np.arange(T * F) % F
    wl_flat = W1[t_idx][:, None] * W2[f_idx, :]
    wr_flat = np.zeros((T * F, T), np.float32)
    wr_flat[np.arange(T * F), t_idx] = W3[f_idx]
    wcat = np.concatenate([wl_flat, wr_flat], axis=1)
    wcat = wcat.reshape(NJ, 128, 2 * T).astype(bf16)
    th2 = np.zeros((128, 2 * K * FO), np.float32)
    for par in range(2):
        for k in range(K):
            th2[par * F:(par + 1) * F,
                par * K * FO + k * FO:(par * K + k + 1) * FO] = Theta[k]
    th2 = th2.astype(bf16)
    hrow = (0.5 * Vs.sum(axis=1)).astype(np.float32).reshape(NCH, 128, 1)
    ident = np.eye(128, dtype=np.float32).astype(bf16)
    return x_tf, bs_t, vs_t, cheb_m, wcat, th2, hrow, ident


def kernel(x, W1, W2, W3, bs, Vs, cheb, Theta, _return_results=False,
           _trace=False, _reps=1):
    from concourse.bass_utils import run_bass_kernel_spmd

    x_tf, bs_t, vs_t, cheb_m, wcat, th2, hrow, ident = _host_prep(
        x, W1, W2, W3, bs, Vs, cheb, Theta)

    key = f"nc{_reps}"
    if key not in _CACHE:
        _CACHE[key] = _build_nc(_reps)
    nc = _CACHE[key]
    _CACHE["nc"] = nc

    shared = dict(bs_t=bs_t, vs_t=vs_t, cheb_m=cheb_m, wcat=wcat,
                  th2=th2, hrow=hrow, ident=ident)
    in_maps = []
    for c in range(NCORES):
        m = dict(shared)
        m["x_tf"] = np.ascontiguousarray(x_tf[c * BPC:(c + 1) * BPC])
        in_maps.append(m)

    _CACHE["in_maps"] = in_maps
    kw = {"trace": True} if _trace else {}
    res = run_bass_kernel_spmd(nc, in_maps, list(range(NCORES)), **kw)
    outs = []
    for c in range(NCORES):
        o = res.results[c]["out"]  # (BPC, 2, NCH, 128, FO, THALF)
        o = o.transpose(0, 2, 3, 4, 1, 5).reshape(BPC, N, FO, T)
        outs.append(o)
    full = np.concatenate(outs, axis=0).astype(np.float32)
    if _return_results:
        return full, res
    return full


# revision 3
# speedup vs baseline: 1.0475x; 1.0475x over previous
"""Trainium2 Bass kernel for the MAMGCN encoder block.

Data-parallel over batch B=16 across 8 NeuronCores (2 batches/core).

Device pipeline per batch:
  x -> attention pre-reductions -> product -> P=tanh(.5(prod+bs))
    -> S_pre=Vs@P -> E=exp(.5 S_pre + hrow) -> colsums -> rT=1/csum
  Y[n,k,t,o] = x @ Theta (block-diag matmul, full T resident)
  conv per m-chunk: A-tiles = cheb_mc * E computed on the fly from an
  m-major cheb layout (one contiguous DMA per m-chunk), accumulate
  po = sum_{k,cn} A^T Y over T-halves, relu(rT*po^T) -> out.

Differences vs the staged baseline:
  * 512-wide PSUM accumulation chains (1 bank/slot) so product/VsP/exp
    pipeline without draining full-1024 tiles; PSUM fits in 8 banks with
    double buffering everywhere.
  * Y built once per batch (full T) with a single strided DVE copy per
    (j,cn) tile instead of two copies.
  * A = cheb*E computed per m-chunk on the fly (no 48KB A buffer), cheb
    streamed in an m-major layout with large contiguous descriptors.
  * x double-buffered in j-halves for cross-batch DMA overlap.
"""
import numpy as np
import ml_dtypes

B, N, F, T, K, FO = 16, 1024, 64, 24, 3, 64
NCORES = 8
BPC = B // NCORES          # batches per core
NCH = N // 128             # 8 partition chunks of N
NJ = (T * F) // 128        # 12 chunks of the tf dim
THALF = T // 2             # 12
bf16 = ml_dtypes.bfloat16

_CACHE = {}


def _build_nc(reps=1, iadd=True, py2=True):
    import contextlib
    import concourse.bacc as bacc
    import concourse.tile as tile
    import concourse.mybir as mybir

    fp32 = mybir.dt.float32
    bf = mybir.dt.bfloat16
    AF = mybir.ActivationFunctionType

    nc = bacc.Bacc(
        "TRN2", target_bir_lowering=False, debug=False,
        num_devices=NCORES,
    )

    # ---- DRAM I/O ----
    x_d = nc.dram_tensor("x_tf", [BPC, NJ, 128, N], bf, kind="ExternalInput")
    bs_d = nc.dram_tensor("bs_t", [NCH, 128, N], bf, kind="ExternalInput")
    vs_d = nc.dram_tensor("vs_t", [NCH, 128, N], bf, kind="ExternalInput")
    # m-major cheb: [mc, 128(n%128), k, cn(n//128), 128(m%128)]
    chebm_d = nc.dram_tensor("cheb_m", [NCH, 128, K * NCH * 128], bf,
                             kind="ExternalInput")
    wcat_d = nc.dram_tensor("wcat", [NJ, 128, 2 * T], bf, kind="ExternalInput")
    th2_d = nc.dram_tensor("th2", [128, 2 * K * FO], bf, kind="ExternalInput")
    hrow_d = nc.dram_tensor("hrow", [NCH, 128, 1], fp32, kind="ExternalInput")
    ident_d = nc.dram_tensor("ident", [128, 128], bf, kind="ExternalInput")
    # out[b, half, mchunk, p, o, tl]
    out_d = nc.dram_tensor("out", [BPC, 2, NCH, 128, FO, THALF], fp32,
                           kind="ExternalOutput")

    with tile.TileContext(nc) as tc:
      with (tc.For_i(0, reps, 1) if reps > 1 else contextlib.nullcontext()):
        with (
            tc.tile_pool(name="const", bufs=1) as cpool,
            tc.tile_pool(name="xp", bufs=1) as xpool,
            tc.tile_pool(name="work", bufs=2) as wpool,
            tc.tile_pool(name="big", bufs=1) as bpool,
            tc.tile_pool(name="psA", bufs=4, space="PSUM") as psA,
            tc.tile_pool(name="psB", bufs=2, space="PSUM") as psB,
        ):
            # ---- constants (~34 KB/part) ----
            vsT_sb = cpool.tile([128, NCH, N], bf, tag="vsT")
            bs_sb = cpool.tile([128, NCH, N], bf, tag="bs")
            wcat_sb = cpool.tile([128, NJ, 2 * T], bf, tag="wcat")
            th2_sb = cpool.tile([128, 2 * K * FO], bf, tag="th2")
            hrow_sb = cpool.tile([128, NCH], fp32, tag="hrow")
            ones_sb = cpool.tile([128, 1], bf, tag="ones")
            one1_sb = cpool.tile([1, 1], fp32, tag="one1")
            ident_sb = cpool.tile([128, 128], bf, tag="ident")
            for j in range(NJ):
                nc.sync.dma_start(wcat_sb[:, j, :], wcat_d[j])
            nc.sync.dma_start(th2_sb[:], th2_d[:])
            nc.sync.dma_start(ident_sb[:], ident_d[:])
            for c in range(NCH):
                nc.sync.dma_start(hrow_sb[:, c:c + 1], hrow_d[c])
            nc.gpsimd.memset(ones_sb[:], 1.0)
            nc.gpsimd.memset(one1_sb[:], 1.0)

            for b in range(BPC):
                # ---- x load in two j-halves (24 KB/part total) ----
                xh = []
                for half in range(2):
                    xt = xpool.tile([128, NJ // 2, N], bf, tag=f"x{half}")
                    for j0 in range(0, NJ // 2, 3):
                        nc.sync.dma_start(
                            xt[:, j0:j0 + 3, :],
                            x_d[b, half * 6 + j0:half * 6 + j0 + 3]
                            .rearrange("j p n -> p j n"))
                    xh.append(xt)
                if b == 0:
                    for c in range(NCH):
                        nc.sync.dma_start(bs_sb[:, c, :], bs_d[c])
                        nc.sync.dma_start(vsT_sb[:, c, :], vs_d[c])

                def xj(j):
                    return xh[j // 6][:, j % 6, :]

                # ---- per-batch big tiles (~104 KB/part) ----
                e_sb = bpool.tile([128, NCH, N], bf, tag="e")
                p_sb = bpool.tile([128, NCH, N], bf, tag="p")
                y_sb = bpool.tile([128, NCH, K, T, FO], bf, tag="y")
                rT_sb = bpool.tile([128, NCH], fp32, tag="rT")

                # ---- attention pre-reductions (one pass over x) ----
                att_c = wpool.tile([2 * T, N], bf, tag="attc", bufs=1)
                att_r = wpool.tile([T, N], bf, tag="attr", bufs=1)
                for s in range(2):
                    pa = psA.tile([2 * T, 512], fp32, tag="A")
                    for j in range(NJ):
                        nc.tensor.matmul(
                            pa[:],
                            wcat_sb[:, j, :],
                            xj(j)[:, s * 512:(s + 1) * 512],
                            start=(j == 0), stop=(j == NJ - 1),
                        )
                    nc.scalar.copy(att_c[:, s * 512:(s + 1) * 512], pa[:])
                # shift rows 24..47 down to partitions 0..23 for the product
                nc.sync.dma_start(att_r[:], att_c[T:2 * T, :])
                att_l = att_c

                # ---- product + bs -> tanh(0.5*) -> P ----
                for cn in range(NCH):
                    for s in range(2):
                        pp = psA.tile([128, 512], fp32, tag="A")
                        nc.tensor.matmul(
                            pp[:],
                            att_l[0:T, cn * 128:(cn + 1) * 128],
                            att_r[:, s * 512:(s + 1) * 512],
                            start=True, stop=not iadd,
                        )
                        if iadd:
                            nc.tensor.matmul(
                                pp[:],
                                ident_sb[:],
                                bs_sb[:, cn, s * 512:(s + 1) * 512],
                                start=False, stop=True,
                            )
                            nc.scalar.activation(
                                p_sb[:, cn, s * 512:(s + 1) * 512],
                                pp[:], AF.Tanh, scale=0.5)
                        else:
                            tmp = wpool.tile([128, 512], bf, tag="tmp")
                            nc.vector.tensor_add(
                                tmp[:], pp[:],
                                bs_sb[:, cn, s * 512:(s + 1) * 512])
                            nc.scalar.activation(
                                p_sb[:, cn, s * 512:(s + 1) * 512],
                                tmp[:], AF.Tanh, scale=0.5)

                # ---- S_pre = Vs @ P (per i-chunk), exp -> E ----
                for ic in range(NCH):
                    for s in range(2):
                        ps = psA.tile([128, 512], fp32, tag="A")
                        for kc in range(NCH):
                            nc.tensor.matmul(
                                ps[:],
                                vsT_sb[:, kc, ic * 128:(ic + 1) * 128],
                                p_sb[:, kc, s * 512:(s + 1) * 512],
                                start=(kc == 0), stop=(kc == NCH - 1),
                            )
                        nc.scalar.activation(
                            e_sb[:, ic, s * 512:(s + 1) * 512], ps[:], AF.Exp,
                            scale=0.5, bias=hrow_sb[:, ic:ic + 1],
                        )

                # ---- column sums of E -> recip -> rT (128, 8) ----
                csum_sb = wpool.tile([1, N], fp32, tag="csum", bufs=1)
                for s in range(2):
                    pc = psA.tile([1, 512], fp32, tag="A")
                    for ic in range(NCH):
                        nc.tensor.matmul(
                            pc[:],
                            ones_sb[:],
                            e_sb[:, ic, s * 512:(s + 1) * 512],
                            start=(ic == 0), stop=(ic == NCH - 1),
                        )
                    nc.scalar.copy(csum_sb[:, s * 512:(s + 1) * 512], pc[:])
                prt = psA.tile([128, NCH], fp32, tag="A")
                for c in range(NCH):
                    nc.tensor.matmul(
                        prt[:, c:c + 1],
                        csum_sb[:, c * 128:(c + 1) * 128],
                        one1_sb[:],
                        start=True, stop=True,
                    )
                nc.vector.reciprocal(rT_sb[:], prt[:])

                # ---- Y build: y[cn, k, t, o] = x @ th2 (full T) ----
                # one strided copy per (j, cn), alternating DVE/ACT
                for j in range(NJ):
                    tl0 = 2 * j
                    for cn in range(NCH):
                        py = psA.tile([128, 2, K, FO], fp32, tag="A")
                        nc.tensor.matmul(
                            py[:, :, :, :],
                            xj(j)[:, cn * 128:(cn + 1) * 128],
                            th2_sb[:],
                            start=True, stop=True,
                        )
                        dst = y_sb[:, cn, :, tl0:tl0 + 2, :]
                        src_ap = py.rearrange("p t k o -> p k t o")
                        if cn % 2 == 0:
                            nc.vector.tensor_copy(dst, src_ap)
                        else:
                            nc.scalar.copy(dst, src_ap)

                # ---- graph conv per m-chunk ----
                for mc in range(NCH):
                    chm = wpool.tile([128, K, NCH, 128], bf, tag="chm")
                    nc.sync.dma_start(
                        chm.rearrange("p k c m -> p (k c m)"), chebm_d[mc])
                    am = wpool.tile([128, K, NCH, 128], bf, tag="am")
                    for k in range(K):
                        nc.vector.tensor_mul(
                            am[:, k, :, :],
                            chm[:, k, :, :],
                            e_sb[:, :, mc * 128:(mc + 1) * 128])
                    for h in range(2):
                        po = psB.tile([128, THALF, FO], fp32, tag="B")
                        nmm = 0
                        for k in range(K):
                            for cn in range(NCH):
                                first = nmm == 0
                                last = nmm == K * NCH - 1
                                t0 = h * THALF
                                nc.tensor.matmul(
                                    po[:, 0:8, :],
                                    am[:, k, cn, :],
                                    y_sb[:, cn, k, t0:t0 + 8, :],
                                    start=first, stop=last,
                                )
                                nc.tensor.matmul(
                                    po[:, 8:THALF, :],
                                    am[:, k, cn, :],
                                    y_sb[:, cn, k, t0 + 8:t0 + THALF, :],
                                    start=first, stop=last,
                                )
                                nmm += 1
                        st = wpool.tile([128, FO, THALF], fp32, tag="stage",
                                        bufs=3)
                        nc.scalar.activation(
                            st[:],
                            po[:, :, :].rearrange("p t o -> p o t"),
                            AF.Relu,
                            scale=rT_sb[:, mc:mc + 1],
                        )
                        nc.sync.dma_start(out_d[b, h, mc], st[:])

    nc.compile()
    return nc


def _host_prep(x, W1, W2, W3, bs, Vs, cheb, Theta):
    x = np.asarray(x, np.float32)
    W1 = np.asarray(W1, np.float32)
    W2 = np.asarray(W2, np.float32)
    W3 = np.asarray(W3, np.float32)
    bs = np.asarray(bs, np.float32)
    Vs = np.asarray(Vs, np.float32)
    cheb = np.asarray(cheb, np.float32)
    Theta = np.asarray(Theta, np.float32)

    x_tf = np.ascontiguousarray(x.transpose(0, 3, 2, 1)).reshape(B, NJ, 128, N)
    x_tf = x_tf.astype(bf16)
    bs_t = bs[0].reshape(NCH, 128, N).astype(bf16)
    vs_t = np.ascontiguousarray(Vs.T).reshape(NCH, 128, N).astype(bf16)
    # m-major cheb: cheb_m[mc, p, k, cn, m] = cheb[k, cn*128+p, mc*128+m]
    cm = cheb.reshape(K, NCH, 128, NCH, 128)          # k, cn, p, mc, m
    cheb_m = np.ascontiguousarray(cm.transpose(3, 2, 0, 1, 4)).reshape(
        NCH, 128, K * NCH * 128).astype(bf16)
    t_idx = np.arange(T * F) // F
    f_idx = np.arange(T * F) % F
    wl_flat = W1[t_idx][:, None] * W2[f_idx, :]
    wr_flat = np.zeros((T * F, T), np.float32)
    wr_flat[np.arange(T * F), t_idx] = W3[f_idx]
    wcat = np.concatenate([wl_flat, wr_flat], axis=1)
    wcat = wcat.reshape(NJ, 128, 2 * T).astype(bf16)
    th2 = np.zeros((128, 2 * K * FO), np.float32)
    for par in range(2):
        for k in range(K):
            th2[par * F:(par + 1) * F,
                par * K * FO + k * FO:(par * K + k + 1) * FO] = Theta[k]
    th2 = th2.astype(bf16)
    hrow = (0.5 * Vs.sum(axis=1)).astype(np.float32).reshape(NCH, 128, 1)
    ident = np.eye(128, dtype=np.float32).astype(bf16)
    return x_tf, bs_t, vs_t, cheb_m, wcat, th2, hrow, ident


def kernel(x, W1, W2, W3, bs, Vs, cheb, Theta, _return_results=False,
           _trace=False, _reps=1):
    from concourse.bass_utils import run_bass_kernel_spmd

    x_tf, bs_t, vs_t, cheb_m, wcat, th2, hrow, ident = _host_prep(
        x, W1, W2, W3, bs, Vs, cheb, Theta)

    key = f"nc{_reps}"
    if key not in _CACHE:
        _CACHE[key] = _build_nc(_reps)
    nc = _CACHE[key]
    _CACHE["nc"] = nc

    shared = dict(bs_t=bs_t, vs_t=vs_t, cheb_m=cheb_m, wcat=wcat,
                  th2=th2, hrow=hrow, ident=ident)
    in_maps = []
    for c in range(NCORES):
        m = dict(shared)
        m["x_tf"] = np.ascontiguousarray(x_tf[c * BPC:(c + 1) * BPC])
        in_maps.append(m)

    _CACHE["in_maps"] = in_maps
    kw = {"trace": True} if _trace else {}
    res = run_bass_kernel_spmd(nc, in_maps, list(range(NCORES)), **kw)
    outs = []
    for c in range(NCORES):
        o = res.results[c]["out"]  # (BPC, 2, NCH, 128, FO, THALF)
        o = o.transpose(0, 2, 3, 4, 1, 5).reshape(BPC, N, FO, T)
        outs.append(o)
    full = np.concatenate(outs, axis=0).astype(np.float32)
    if _return_results:
        return full, res
    return full


# revision 4
# speedup vs baseline: 1.0753x; 1.0265x over previous
"""Trainium2 Bass kernel for the MAMGCN encoder block.

Data-parallel over batch B=16 across 8 NeuronCores (2 batches/core).

Device pipeline per batch:
  x -> attention pre-reductions -> product -> P=tanh(.5(prod+bs))
    -> S_pre=Vs@P -> E=exp(.5 S_pre + hrow) -> colsums -> rT=1/csum
  Y[n,k,t,o] = x @ Theta (block-diag matmul, full T resident)
  conv per m-chunk: A-tiles = cheb_mc * E computed on the fly from an
  m-major cheb layout (one contiguous DMA per m-chunk), accumulate
  po = sum_{k,cn} A^T Y over T-halves, relu(rT*po^T) -> out.

Differences vs the staged baseline:
  * 512-wide PSUM accumulation chains (1 bank/slot) so product/VsP/exp
    pipeline without draining full-1024 tiles; PSUM fits in 8 banks with
    double buffering everywhere.
  * Y built once per batch (full T) with a single strided DVE copy per
    (j,cn) tile instead of two copies.
  * A = cheb*E computed per m-chunk on the fly (no 48KB A buffer), cheb
    streamed in an m-major layout with large contiguous descriptors.
  * x double-buffered in j-halves for cross-batch DMA overlap.
"""
import numpy as np
import ml_dtypes

B, N, F, T, K, FO = 16, 1024, 64, 24, 3, 64
NCORES = 8
BPC = B // NCORES          # batches per core
NCH = N // 128             # 8 partition chunks of N
NJ = (T * F) // 128        # 12 chunks of the tf dim
THALF = T // 2             # 12
bf16 = ml_dtypes.bfloat16

_CACHE = {}


def _build_nc(reps=1, iadd=True, py2=True):
    import contextlib
    import concourse.bacc as bacc
    import concourse.tile as tile
    import concourse.mybir as mybir

    fp32 = mybir.dt.float32
    bf = mybir.dt.bfloat16
    AF = mybir.ActivationFunctionType

    nc = bacc.Bacc(
        "TRN2", target_bir_lowering=False, debug=False,
        num_devices=NCORES,
    )

    # ---- DRAM I/O ----
    x_d = nc.dram_tensor("x_tf", [BPC, 2, 128, (NJ // 2) * N], bf,
                         kind="ExternalInput")
    bs_d = nc.dram_tensor("bs_t", [NCH, 128, N], bf, kind="ExternalInput")
    vs_d = nc.dram_tensor("vs_t", [NCH, 128, N], bf, kind="ExternalInput")
    # m-major cheb: [mc, 128(n%128), k, cn(n//128), 128(m%128)]
    chebm_d = nc.dram_tensor("cheb_m", [NCH, 128, K * NCH * 128], bf,
                             kind="ExternalInput")
    wcat_d = nc.dram_tensor("wcat", [NJ, 128, 2 * T], bf, kind="ExternalInput")
    th2_d = nc.dram_tensor("th2", [128, 2 * K * FO], bf, kind="ExternalInput")
    hrow_d = nc.dram_tensor("hrow", [NCH, 128, 1], fp32, kind="ExternalInput")
    ident_d = nc.dram_tensor("ident", [128, 128], bf, kind="ExternalInput")
    # out[b, half, mchunk, p, o, tl]
    out_d = nc.dram_tensor("out", [BPC, 2, NCH, 128, FO, THALF], fp32,
                           kind="ExternalOutput")

    with tile.TileContext(nc) as tc:
      with (tc.For_i(0, reps, 1) if reps > 1 else contextlib.nullcontext()):
        with (
            tc.tile_pool(name="const", bufs=1) as cpool,
            tc.tile_pool(name="xp", bufs=1) as xpool,
            tc.tile_pool(name="work", bufs=2) as wpool,
            tc.tile_pool(name="big", bufs=1) as bpool,
            tc.tile_pool(name="psA", bufs=4, space="PSUM") as psA,
            tc.tile_pool(name="psB", bufs=2, space="PSUM") as psB,
        ):
            # ---- constants (~34 KB/part) ----
            vsT_sb = cpool.tile([128, NCH, N], bf, tag="vsT")
            bs_sb = cpool.tile([128, NCH, N], bf, tag="bs")
            wcat_sb = cpool.tile([128, NJ, 2 * T], bf, tag="wcat")
            th2_sb = cpool.tile([128, 2 * K * FO], bf, tag="th2")
            hrow_sb = cpool.tile([128, NCH], fp32, tag="hrow")
            ones_sb = cpool.tile([128, 1], bf, tag="ones")
            one1_sb = cpool.tile([1, 1], fp32, tag="one1")
            ident_sb = cpool.tile([128, 128], bf, tag="ident")
            for j in range(NJ):
                nc.sync.dma_start(wcat_sb[:, j, :], wcat_d[j])
            nc.sync.dma_start(th2_sb[:], th2_d[:])
            nc.sync.dma_start(ident_sb[:], ident_d[:])
            for c in range(NCH):
                nc.sync.dma_start(hrow_sb[:, c:c + 1], hrow_d[c])
            nc.gpsimd.memset(ones_sb[:], 1.0)
            nc.gpsimd.memset(one1_sb[:], 1.0)

            for b in range(BPC):
                # ---- x load in two j-halves (24 KB/part total) ----
                xh = []
                for half in range(2):
                    xt = xpool.tile([128, NJ // 2, N], bf, tag=f"x{half}")
                    nc.sync.dma_start(
                        xt.rearrange("p j n -> p (j n)"), x_d[b, half])
                    xh.append(xt)
                if b == 0:
                    for c in range(NCH):
                        nc.sync.dma_start(bs_sb[:, c, :], bs_d[c])
                        nc.sync.dma_start(vsT_sb[:, c, :], vs_d[c])

                def xj(j):
                    return xh[j // 6][:, j % 6, :]

                # ---- per-batch big tiles (~104 KB/part) ----
                e_sb = bpool.tile([128, NCH, N], bf, tag="e")
                p_sb = bpool.tile([128, NCH, N], bf, tag="p")
                y_sb = bpool.tile([128, NCH, K, T, FO], bf, tag="y")
                rT_sb = bpool.tile([128, NCH], fp32, tag="rT")

                # ---- attention pre-reductions (one pass over x) ----
                att_c = wpool.tile([2 * T, N], bf, tag="attc", bufs=1)
                att_r = wpool.tile([T, N], bf, tag="attr", bufs=1)
                for s in range(2):
                    pa = psA.tile([2 * T, 512], fp32, tag="A")
                    for j in range(NJ):
                        nc.tensor.matmul(
                            pa[:],
                            wcat_sb[:, j, :],
                            xj(j)[:, s * 512:(s + 1) * 512],
                            start=(j == 0), stop=(j == NJ - 1),
                        )
                    nc.scalar.copy(att_c[:, s * 512:(s + 1) * 512], pa[:])
                # shift rows 24..47 down to partitions 0..23 for the product
                nc.sync.dma_start(att_r[:], att_c[T:2 * T, :])
                att_l = att_c

                # ---- product + bs -> tanh(0.5*) -> P ----
                for cn in range(NCH):
                    for s in range(2):
                        pp = psA.tile([128, 512], fp32, tag="A")
                        nc.tensor.matmul(
                            pp[:],
                            att_l[0:T, cn * 128:(cn + 1) * 128],
                            att_r[:, s * 512:(s + 1) * 512],
                            start=True, stop=not iadd,
                        )
                        if iadd:
                            nc.tensor.matmul(
                                pp[:],
                                ident_sb[:],
                                bs_sb[:, cn, s * 512:(s + 1) * 512],
                                start=False, stop=True,
                            )
                            nc.scalar.activation(
                                p_sb[:, cn, s * 512:(s + 1) * 512],
                                pp[:], AF.Tanh, scale=0.5)
                        else:
                            tmp = wpool.tile([128, 512], bf, tag="tmp")
                            nc.vector.tensor_add(
                                tmp[:], pp[:],
                                bs_sb[:, cn, s * 512:(s + 1) * 512])
                            nc.scalar.activation(
                                p_sb[:, cn, s * 512:(s + 1) * 512],
                                tmp[:], AF.Tanh, scale=0.5)

                # ---- S_pre = Vs @ P (per i-chunk), exp -> E ----
                for ic in range(NCH):
                    for s in range(2):
                        ps = psA.tile([128, 512], fp32, tag="A")
                        for kc in range(NCH):
                            nc.tensor.matmul(
                                ps[:],
                                vsT_sb[:, kc, ic * 128:(ic + 1) * 128],
                                p_sb[:, kc, s * 512:(s + 1) * 512],
                                start=(kc == 0), stop=(kc == NCH - 1),
                            )
                        nc.scalar.activation(
                            e_sb[:, ic, s * 512:(s + 1) * 512], ps[:], AF.Exp,
                            scale=0.5, bias=hrow_sb[:, ic:ic + 1],
                        )

                # ---- column sums of E -> recip -> rT (128, 8) ----
                csum_sb = wpool.tile([1, N], fp32, tag="csum", bufs=1)
                for s in range(2):
                    pc = psA.tile([1, 512], fp32, tag="A")
                    for ic in range(NCH):
                        nc.tensor.matmul(
                            pc[:],
                            ones_sb[:],
                            e_sb[:, ic, s * 512:(s + 1) * 512],
                            start=(ic == 0), stop=(ic == NCH - 1),
                        )
                    nc.scalar.copy(csum_sb[:, s * 512:(s + 1) * 512], pc[:])
                prt = psA.tile([128, NCH], fp32, tag="A")
                for c in range(NCH):
                    nc.tensor.matmul(
                        prt[:, c:c + 1],
                        csum_sb[:, c * 128:(c + 1) * 128],
                        one1_sb[:],
                        start=True, stop=True,
                    )
                nc.vector.reciprocal(rT_sb[:], prt[:])

                # ---- Y build: y[cn, k, t, o] = x @ th2 (full T) ----
                # one strided copy per (j, cn), alternating DVE/ACT
                for j in range(NJ):
                    tl0 = 2 * j
                    for cn in range(NCH):
                        py = psA.tile([128, 2, K, FO], fp32, tag="A")
                        nc.tensor.matmul(
                            py[:, :, :, :],
                            xj(j)[:, cn * 128:(cn + 1) * 128],
                            th2_sb[:],
                            start=True, stop=True,
                        )
                        dst = y_sb[:, cn, :, tl0:tl0 + 2, :]
                        src_ap = py.rearrange("p t k o -> p k t o")
                        if cn % 2 == 0:
                            nc.vector.tensor_copy(dst, src_ap)
                        else:
                            nc.scalar.copy(dst, src_ap)

                # ---- graph conv per m-chunk ----
                for mc in range(NCH):
                    chm = wpool.tile([128, K, NCH, 128], bf, tag="chm",
                                     bufs=3)
                    nc.sync.dma_start(
                        chm.rearrange("p k c m -> p (k c m)"), chebm_d[mc])
                    am = wpool.tile([128, K, NCH, 128], bf, tag="am")
                    for k in range(K):
                        nc.vector.tensor_mul(
                            am[:, k, :, :],
                            chm[:, k, :, :],
                            e_sb[:, :, mc * 128:(mc + 1) * 128])
                    for h in range(2):
                        po = psB.tile([128, THALF, FO], fp32, tag="B")
                        nmm = 0
                        for k in range(K):
                            for cn in range(NCH):
                                first = nmm == 0
                                last = nmm == K * NCH - 1
                                t0 = h * THALF
                                nc.tensor.matmul(
                                    po[:, 0:8, :],
                                    am[:, k, cn, :],
                                    y_sb[:, cn, k, t0:t0 + 8, :],
                                    start=first, stop=last,
                                )
                                nc.tensor.matmul(
                                    po[:, 8:THALF, :],
                                    am[:, k, cn, :],
                                    y_sb[:, cn, k, t0 + 8:t0 + THALF, :],
                                    start=first, stop=last,
                                )
                                nmm += 1
                        st = wpool.tile([128, FO, THALF], fp32, tag="stage",
                                        bufs=2)
                        nc.scalar.activation(
                            st[:],
                            po[:, :, :].rearrange("p t o -> p o t"),
                            AF.Relu,
                            scale=rT_sb[:, mc:mc + 1],
                        )
                        nc.sync.dma_start(out_d[b, h, mc], st[:])

    nc.compile()
    return nc


def _host_prep(x, W1, W2, W3, bs, Vs, cheb, Theta):
    x = np.asarray(x, np.float32)
    W1 = np.asarray(W1, np.float32)
    W2 = np.asarray(W2, np.float32)
    W3 = np.asarray(W3, np.float32)
    bs = np.asarray(bs, np.float32)
    Vs = np.asarray(Vs, np.float32)
    cheb = np.asarray(cheb, np.float32)
    Theta = np.asarray(Theta, np.float32)

    x_tf = np.ascontiguousarray(x.transpose(0, 3, 2, 1)).reshape(B, NJ, 128, N)
    x_tf = np.ascontiguousarray(
        x_tf.reshape(B, 2, NJ // 2, 128, N).transpose(0, 1, 3, 2, 4)
    ).reshape(B, 2, 128, (NJ // 2) * N).astype(bf16)
    bs_t = bs[0].reshape(NCH, 128, N).astype(bf16)
    vs_t = np.ascontiguousarray(Vs.T).reshape(NCH, 128, N).astype(bf16)
    # m-major cheb: cheb_m[mc, p, k, cn, m] = cheb[k, cn*128+p, mc*128+m]
    cm = cheb.reshape(K, NCH, 128, NCH, 128)          # k, cn, p, mc, m
    cheb_m = np.ascontiguousarray(cm.transpose(3, 2, 0, 1, 4)).reshape(
        NCH, 128, K * NCH * 128).astype(bf16)
    t_idx = np.arange(T * F) // F
    f_idx = np.arange(T * F) % F
    wl_flat = W1[t_idx][:, None] * W2[f_idx, :]
    wr_flat = np.zeros((T * F, T), np.float32)
    wr_flat[np.arange(T * F), t_idx] = W3[f_idx]
    wcat = np.concatenate([wl_flat, wr_flat], axis=1)
    wcat = wcat.reshape(NJ, 128, 2 * T).astype(bf16)
    th2 = np.zeros((128, 2 * K * FO), np.float32)
    for par in range(2):
        for k in range(K):
            th2[par * F:(par + 1) * F,
                par * K * FO + k * FO:(par * K + k + 1) * FO] = Theta[k]
    th2 = th2.astype(bf16)
    hrow = (0.5 * Vs.sum(axis=1)).astype(np.float32).reshape(NCH, 128, 1)
    ident = np.eye(128, dtype=np.float32).astype(bf16)
    return x_tf, bs_t, vs_t, cheb_m, wcat, th2, hrow, ident


def kernel(x, W1, W2, W3, bs, Vs, cheb, Theta, _return_results=False,
           _trace=False, _reps=1):
    from concourse.bass_utils import run_bass_kernel_spmd

    x_tf, bs_t, vs_t, cheb_m, wcat, th2, hrow, ident = _host_prep(
        x, W1, W2, W3, bs, Vs, cheb, Theta)

    key = f"nc{_reps}"
    if key not in _CACHE:
        _CACHE[key] = _build_nc(_reps)
    nc = _CACHE[key]
    _CACHE["nc"] = nc

    shared = dict(bs_t=bs_t, vs_t=vs_t, cheb_m=cheb_m, wcat=wcat,
                  th2=th2, hrow=hrow, ident=ident)
    in_maps = []
    for c in range(NCORES):
        m = dict(shared)
        m["x_tf"] = np.ascontiguousarray(x_tf[c * BPC:(c + 1) * BPC])
        in_maps.append(m)

    _CACHE["in_maps"] = in_maps
    kw = {"trace": True} if _trace else {}
    res = run_bass_kernel_spmd(nc, in_maps, list(range(NCORES)), **kw)
    outs = []
    for c in range(NCORES):
        o = res.results[c]["out"]  # (BPC, 2, NCH, 128, FO, THALF)
        o = o.transpose(0, 2, 3, 4, 1, 5).reshape(BPC, N, FO, T)
        outs.append(o)
    full = np.concatenate(outs, axis=0).astype(np.float32)
    if _return_results:
        return full, res
    return full
